# revision 1
# baseline (speedup 1.0000x reference)
"""Trainium2 Bass kernel for a DEC-style clustering loss (loss_fn).

loss = mean((X - decoding)^2) + ALPHA * KL
where KL is the batch-mean KL(p || q) of the Student-t (1 dof) soft
assignment q of `encoding` rows to `centres`, against the (detached)
DEC target distribution p = rownorm(q^2 / f), f_j = sum_i q_ij.

Distribution: data-parallel over the N=200000 rows across 8 NeuronCores
(25000 rows/core); centres replicated. The only cross-core coupling is
f (100 floats, AllReduce on-device) and the final scalar partial sums
(summed on host).

Math (per core, rows i, clusters j):
  u_ij = 1/(1 + d2_ij),  d2 = |e_i|^2 + |c_j|^2 - 2 e.c   (>= ~25 here,
        so the reference's max(d2,0) clamp can never bind)
  s_i = sum_j u_ij,  q_ij = u_ij/s_i,  f_j = sum_i q_ij  (AllReduce)
  w_ij = u_ij^2/f_j = s_i^2 * (q^2/f)_ij,  r_i = sum_j w_ij
  p_ij = w_ij / r_i          (the s_i^2 cancels in the row-normalize)
  log p - log q = log(u/f) - log r + log s
  KL*N = sum_i [ (sum_j w_ij*log(u_ij/f_j))/r_i - log r_i + log s_i ]

Precision: everything fp32 except the stored u (fp16; verified to move
the final loss by <1e-4 relative — bf16 anywhere in the KL chain is
catastrophic because KL ~ 5e-4 comes from cancelling O(10) logs).
"""

import os

import numpy as np

import concourse.bass as bass
import concourse.mybir as mybir
import concourse.tile as tile
from concourse import masks
from concourse.bass_utils import run_bass_kernel_spmd

F32 = mybir.dt.float32
F16 = mybir.dt.float16
AF = mybir.ActivationFunctionType
ALU = mybir.AluOpType
AX = mybir.AxisListType

N, D_IN, D_LAT, K = 200000, 256, 64, 100
ALPHA = 1000.0
NCORES = 8
NROWS = N // NCORES            # 25000 rows per core
G = 4                          # row-groups per supertile
GP = 125                       # rows per group (partition dim)
ST = G * GP                    # 500 rows per supertile
NST = NROWS // ST              # 50 supertiles
# MSE stream: per-core flat view [128, 50000] of both X and decoding
MSE_P = 128
MSE_FD = 2000
MSE_T = (NROWS * D_IN) // (MSE_P * MSE_FD)   # 25 tiles


def _emit_mse_tile(nc, mse_pool, sse_stage, xv, dv, t):
    """One [128, MSE_FD] tile of sum((X-dec)^2): DMA loads, gpsimd
    subtract, ACT square with fused per-partition accumulation."""
    xt = mse_pool.tile([MSE_P, MSE_FD], F32, tag="mse_x")
    dt = mse_pool.tile([MSE_P, MSE_FD], F32, tag="mse_d")
    nc.sync.dma_start(out=xt[:], in_=xv[:, t * MSE_FD:(t + 1) * MSE_FD])
    nc.sync.dma_start(out=dt[:], in_=dv[:, t * MSE_FD:(t + 1) * MSE_FD])
    diff = mse_pool.tile([MSE_P, MSE_FD], F32, tag="mse_diff")
    nc.gpsimd.tensor_tensor(diff[:], xt[:], dt[:], ALU.subtract)
    sq = mse_pool.tile([MSE_P, MSE_FD], F32, tag="mse_sq")
    nc.scalar.activation(sq[:], diff[:], AF.Square,
                         accum_out=sse_stage[:, t:t + 1])


def _body(tc, ctx, x_in, d_in, e_in, c_in, out, parts=("mse", "a", "b")):
    nc = tc.nc
    singles = ctx.enter_context(tc.tile_pool(name="singles", bufs=1))
    a_pool = ctx.enter_context(tc.tile_pool(name="passa", bufs=3))
    b_pool = ctx.enter_context(tc.tile_pool(name="passb", bufs=3))
    mse_pool = ctx.enter_context(tc.tile_pool(name="mse", bufs=2))
    ps_pool = ctx.enter_context(tc.tile_pool(name="ps", bufs=2, space="PSUM"))
    ps1_pool = ctx.enter_context(tc.tile_pool(name="ps1", bufs=1, space="PSUM"))
    dram = ctx.enter_context(tc.tile_pool(name="dram", bufs=1, space="DRAM"))

    # ---------------- one-time constants ----------------
    identity = singles.tile([128, 128], F32)
    masks.make_identity(nc, identity[:])

    c_sb = singles.tile([K, D_LAT], F32)
    nc.sync.dma_start(out=c_sb[:], in_=c_in.ap())
    # c2_j = sum_d centres^2 (per-partition accumulate of Square)
    c_sq = singles.tile([K, D_LAT], F32)
    c2p1 = singles.tile([K, 1], F32)
    c2 = singles.tile([K, 1], F32)
    nc.scalar.activation(c_sq[:], c_sb[:], AF.Square, accum_out=c2[:])
    nc.vector.tensor_scalar_add(c2p1[:], c2[:], 1.0)
    cm2 = singles.tile([K, D_LAT], F32)   # -2 * centres
    nc.scalar.activation(cm2[:], c_sb[:], AF.Copy, scale=-2.0)
    # rhs_aug [66, 100]: rows 0..63 = -2*C^T, row 64 = (c2+1)^T,
    # row 65 = ones. It pairs with per-group lhsT = [e^T; ones; s_e^T]
    # so ONE matmul per group yields d2+1 = -2e.c + (c2+1) + |e|^2.
    cmT_ps = ps_pool.tile([D_LAT, K], F32, tag="eT_ps")
    nc.tensor.transpose(cmT_ps[:], cm2[:], identity[:K, :K])
    c2T_ps = ps_pool.tile([1, K], F32, tag="seT_ps")
    nc.tensor.transpose(c2T_ps[:], c2p1[:], identity[:K, :K])
    c2T_sb = singles.tile([1, K], F32)
    nc.scalar.copy(c2T_sb[:], c2T_ps[:])
    rhs_aug = singles.tile([D_LAT + 2, K], F32)
    nc.scalar.copy(rhs_aug[0:D_LAT, :], cmT_ps[:])
    nc.vector.memset(rhs_aug[D_LAT:D_LAT + 2, :], 1.0)
    # cross-partition move (part 0 -> part 64) via SBUF-to-SBUF DMA
    nc.sync.dma_start(out=rhs_aug[D_LAT:D_LAT + 1, :], in_=c2T_sb[:])

    ones_col = singles.tile([128, 1], F32)
    nc.vector.memset(ones_col[:], 1.0)

    # persistent stages
    u_store = singles.tile([GP, NST * G * K], F16)        # 40KB/part
    s_stage = singles.tile([GP, NST * G], F32)
    r_stage = singles.tile([GP, NST * G], F32)
    a_stage = singles.tile([GP, NST * G], F32)
    sse_stage = singles.tile([MSE_P, MSE_T], F32)

    f_ps = ps1_pool.tile([1, K], F32, tag="f_ps")

    # flat views for the MSE stream
    xv = x_in.ap().rearrange("a b -> (a b)").rearrange("(p f) -> p f", p=MSE_P)
    dv = d_in.ap().rearrange("a b -> (a b)").rearrange("(p f) -> p f", p=MSE_P)
    # encoding view: supertile st, partition p, free (g, d)
    ev = e_in.ap().rearrange("(s g p) d -> s p g d", g=G, p=GP)

    # ---------------- pass A + interleaved MSE ----------------
    GW = D_LAT + 2    # 66 columns per group: [e (64) | ones | s_e]
    for st in range(NST if "a" in parts else 0):
        # e lands in per-group 66-column blocks, leaving room for the
        # ones and |e|^2 columns that ride along into the transpose.
        e_t = a_pool.tile([GP, G * GW], F32, tag="e_t")
        e_view = bass.AP(
            tensor=e_t[:].tensor, offset=e_t[:].offset,
            ap=[list(e_t[:].ap[0]), [GW, G], [1, D_LAT]])
        nc.sync.dma_start(out=e_view, in_=ev[st])
        ones_cols = bass.AP(
            tensor=e_t[:].tensor, offset=e_t[:].offset + D_LAT,
            ap=[list(e_t[:].ap[0]), [GW, G], [1, 1]])
        nc.vector.memset(ones_cols, 1.0)
        # |e|^2 per row-group: square then blocked reduce into col 65
        e2 = a_pool.tile([GP, G * D_LAT], F32, tag="e2")
        nc.scalar.activation(
            e2[:].rearrange("p (g d) -> p g d", g=G), e_view, AF.Square)
        se_cols = bass.AP(
            tensor=e_t[:].tensor, offset=e_t[:].offset + D_LAT + 1,
            ap=[list(e_t[:].ap[0]), [GW, G], [1, 1]])
        nc.vector.tensor_reduce(
            se_cols, e2[:].rearrange("p (g d) -> p g d", g=G),
            axis=AX.X, op=ALU.add)

        # per group: transpose [e|1|s_e] -> lhsT [66,125]; one matmul
        # with rhs_aug gives psum = -2e.c + (c2+1) + |e|^2 = d2+1
        dot_ps = ps_pool.tile([GP, G * K], F32, tag="dot_ps")
        for g in range(G):
            eTa_ps = ps_pool.tile([GW, GP], F32, tag="eT_ps")
            nc.tensor.transpose(
                eTa_ps[:], e_t[:, g * GW:(g + 1) * GW], identity[:GP, :GP])
            eTa = a_pool.tile([GW, GP], F32, tag=f"eTa{g % 2}")
            nc.scalar.copy(eTa[:], eTa_ps[:])
            nc.tensor.matmul(dot_ps[:, g * K:(g + 1) * K], eTa[:],
                             rhs_aug[:], start=True, stop=True)
        # u = 1/(d2+1)
        u32 = a_pool.tile([GP, G * K], F32, tag="u32")
        nc.vector.reciprocal_approx_fast(u32[:], dot_ps[:])
        # fp16 copy for storage (gpsimd, keeps DVE free)
        nc.gpsimd.tensor_copy(u_store[:, st * G * K:(st + 1) * G * K], u32[:])
        # s = sum_j u per group
        nc.vector.tensor_reduce(
            s_stage[:, st * G:(st + 1) * G],
            u32[:].rearrange("p (g k) -> p g k", g=G),
            axis=AX.X, op=ALU.add)
        rs = a_pool.tile([GP, G], F32, tag="rs")
        nc.vector.reciprocal_approx_fast(
            rs[:], s_stage[:, st * G:(st + 1) * G])
        # f += sum_i u_ij / s_i  (PSUM accumulation across all supertiles)
        for g in range(G):
            nc.tensor.matmul(f_ps[:], rs[:, g:g + 1],
                             u32[:, g * K:(g + 1) * K],
                             start=(st == 0 and g == 0),
                             stop=(st == NST - 1 and g == G - 1))
        if "mse" in parts and st % 2 == 0 and st // 2 < MSE_T:
            _emit_mse_tile(nc, mse_pool, sse_stage, xv, dv, st // 2)

    if "mse" in parts:
        for t in range(NST // 2 if "a" in parts else 0, MSE_T):
            _emit_mse_tile(nc, mse_pool, sse_stage, xv, dv, t)
    else:
        nc.vector.memset(sse_stage[:], 0.0)

    if "a" not in parts:
        for t in [s_stage, r_stage, a_stage]:
            nc.vector.memset(t[:], 1.0)
        nc.vector.memset(u_store[:], 1.0)

    # ---------------- AllReduce f ----------------
    do_b = "a" in parts and "b" in parts
    do_cc = "a" in parts and "nocc" not in parts
    f_sb = singles.tile([1, K], F32)
    if "a" in parts:
        nc.scalar.copy(f_sb[:], f_ps[:])
    else:
        nc.vector.memset(f_sb[:], 1.0)
    f_part_dram = dram.tile([1, K], F32)
    f_all_dram = dram.tile([1, K], F32)
    nc.sync.dma_start(out=f_part_dram[:], in_=f_sb[:])
    if do_cc:
        nc.gpsimd.collective_compute(
            "AllReduce", ALU.add,
            replica_groups=[list(range(NCORES))],
            ins=[f_part_dram[:].opt()],
            outs=[f_all_dram[:].opt()],
        )
    else:
        nc.sync.dma_start(out=f_all_dram[:], in_=f_part_dram[:])
    # broadcast-load f to all partitions, then 1/f
    f_rep = singles.tile([GP, K], F32)
    f_ap = f_all_dram[:]
    f_bcast_ap = bass.AP(
        tensor=f_ap.tensor, offset=f_ap.offset,
        ap=[[0, GP]] + [list(d) for d in f_ap.ap[1:]],
    )
    nc.sync.dma_start(out=f_rep[:], in_=f_bcast_ap)
    # finv is scaled by S=2^17 so that ln(u*finv) ~ 0. The scale cancels
    # exactly in a/r - ln r, but keeps a ~ 0 so the ~2e-6 noise of the
    # fast reciprocal of r is not amplified by |ln v| ~ 12 (which would
    # otherwise bias the tiny KL by ~5%).
    finv = singles.tile([GP, K], F32)
    nc.vector.reciprocal_approx_fast(finv[:], f_rep[:])
    nc.vector.tensor_scalar_mul(finv[:], finv[:], float(2.0 ** 17))

    # ---------------- pass B (all fp32 except the fp16 u read) --------
    if not do_b and "a" in parts:
        nc.vector.memset(r_stage[:], 1.0)
        nc.vector.memset(a_stage[:], 1.0)
    for st in range(NST if do_b else 0):
        ub = u_store[:, st * G * K:(st + 1) * G * K]
        v = b_pool.tile([GP, G * K], F32, tag="v")
        for g in range(G):
            nc.vector.tensor_tensor(
                v[:, g * K:(g + 1) * K], ub[:, g * K:(g + 1) * K],
                finv[:], ALU.mult)
        lv = b_pool.tile([GP, G * K], F32, tag="lv")
        nc.scalar.activation(lv[:], v[:], AF.Ln)
        w = b_pool.tile([GP, G * K], F32, tag="w")
        nc.vector.tensor_tensor(w[:], v[:], ub, ALU.mult)
        nc.vector.tensor_reduce(
            r_stage[:, st * G:(st + 1) * G],
            w[:].rearrange("p (g k) -> p g k", g=G),
            axis=AX.X, op=ALU.add)
        z = b_pool.tile([GP, G * K], F32, tag="z")
        nc.vector.tensor_tensor(z[:], w[:], lv[:], ALU.mult)
        nc.vector.tensor_reduce(
            a_stage[:, st * G:(st + 1) * G],
            z[:].rearrange("p (g k) -> p g k", g=G),
            axis=AX.X, op=ALU.add)

    # ---------------- final combine ----------------
    rr = singles.tile([GP, NST * G], F32)
    nc.vector.reciprocal_approx_fast(rr[:], r_stage[:])
    y = singles.tile([GP, NST * G], F32)
    nc.vector.tensor_tensor(y[:], a_stage[:], rr[:], ALU.mult)
    lnr = singles.tile([GP, NST * G], F32)
    nc.scalar.activation(lnr[:], r_stage[:], AF.Ln)
    lns = singles.tile([GP, NST * G], F32)
    nc.scalar.activation(lns[:], s_stage[:], AF.Ln)
    t1 = singles.tile([GP, NST * G], F32)
    nc.vector.tensor_tensor(t1[:], y[:], lnr[:], ALU.subtract)
    t2 = singles.tile([GP, NST * G], F32)
    nc.vector.tensor_tensor(t2[:], t1[:], lns[:], ALU.add)
    kcol = singles.tile([GP, 1], F32)
    nc.vector.tensor_reduce(kcol[:], t2[:], axis=AX.X, op=ALU.add)
    ssecol = singles.tile([MSE_P, 1], F32)
    nc.vector.tensor_reduce(ssecol[:], sse_stage[:], axis=AX.X, op=ALU.add)
    # partition reduction via matmul with a ones column
    red_ps = ps_pool.tile([1, 2], F32, tag="dot_ps")
    nc.tensor.matmul(red_ps[:, 0:1], ssecol[:], ones_col[:MSE_P, :],
                     start=True, stop=True)
    nc.tensor.matmul(red_ps[:, 1:2], kcol[:], ones_col[:GP, :],
                     start=True, stop=True)
    out_sb = singles.tile([1, 2], F32)
    nc.scalar.copy(out_sb[:], red_ps[:])
    nc.sync.dma_start(out=out.ap(), in_=out_sb[:])


def build_kernel(parts=("mse", "a", "b")):
    from contextlib import ExitStack
    import concourse.bacc as bacc
    nc = bacc.Bacc("TRN2", target_bir_lowering=False, debug=False,
                   num_devices=NCORES)
    x_in = nc.dram_tensor("x_shard", [NROWS, D_IN], F32, kind="ExternalInput")
    d_in = nc.dram_tensor("dec_shard", [NROWS, D_IN], F32,
                          kind="ExternalInput")
    e_in = nc.dram_tensor("enc_shard", [NROWS, D_LAT], F32,
                          kind="ExternalInput")
    c_in = nc.dram_tensor("centres", [K, D_LAT], F32, kind="ExternalInput")
    out = nc.dram_tensor("partials", [1, 2], F32, kind="ExternalOutput")
    with tile.TileContext(nc) as tc:
        with ExitStack() as ctx:
            _body(tc, ctx, x_in, d_in, e_in, c_in, out, parts=parts)
    nc.compile()
    return nc


_built = None
LAST_RESULTS = None


def _get_built():
    global _built
    if _built is None:
        parts = tuple(os.environ.get("KERNEL_PARTS", "mse,a,b").split(","))
        _built = build_kernel(parts)
    return _built


def kernel(X, encoding, decoding, centres):
    X = np.ascontiguousarray(np.asarray(X, dtype=np.float32))
    encoding = np.ascontiguousarray(np.asarray(encoding, dtype=np.float32))
    decoding = np.ascontiguousarray(np.asarray(decoding, dtype=np.float32))
    centres = np.ascontiguousarray(np.asarray(centres, dtype=np.float32))

    nc = _get_built()
    in_maps = []
    for c in range(NCORES):
        sl = slice(c * NROWS, (c + 1) * NROWS)
        in_maps.append({
            "x_shard": X[sl],
            "dec_shard": decoding[sl],
            "enc_shard": encoding[sl],
            "centres": centres,
        })
    res = run_bass_kernel_spmd(nc, in_maps, core_ids=list(range(NCORES)))
    global LAST_RESULTS
    LAST_RESULTS = res
    sse = 0.0
    kl_sum = 0.0
    for r in res.results:
        sse += float(r["partials"][0, 0])
        kl_sum += float(r["partials"][0, 1])
    loss = sse / (N * D_IN) + ALPHA * (kl_sum / N)
    return np.float32(loss)



# revision 6
# speedup vs baseline: 1.1376x; 1.1376x over previous
"""Trainium2 Bass kernel for a DEC-style clustering loss (loss_fn).

loss = mean((X - decoding)^2) + ALPHA * KL
where KL is the batch-mean KL(p || q) of the Student-t (1 dof) soft
assignment q of `encoding` rows to `centres`, against the (detached)
DEC target distribution p = rownorm(q^2 / f), f_j = sum_i q_ij.

Distribution: data-parallel over the N=200000 rows across 8 NeuronCores
(25000 rows/core); centres replicated. The only cross-core coupling is
f (100 floats, AllReduce on-device) and the final scalar partial sums
(summed on host).

Math (per core, rows i, clusters j):
  u_ij = 1/(1 + d2_ij),  d2 = |e_i|^2 + |c_j|^2 - 2 e.c   (>= ~25 here,
        so the reference's max(d2,0) clamp can never bind)
  s_i = sum_j u_ij,  q_ij = u_ij/s_i,  f_j = sum_i q_ij  (AllReduce)
  w_ij = u_ij^2/f_j = s_i^2 * (q^2/f)_ij,  r_i = sum_j w_ij
  p_ij = w_ij / r_i          (the s_i^2 cancels in the row-normalize)
  log p - log q = log(u/f) - log r + log s
  KL*N = sum_i [ (sum_j w_ij*log(u_ij/f_j))/r_i - log r_i + log s_i ]

v2 layout/perf notes vs the original:
  - All PE operands are fp16 (fp32 matmuls stream 4x slower); the four
    per-group transposes write one [66,500] PSUM tile copied once.
  - The per-element MSE subtract is done by the DMA engine itself
    (SWDGE accum_op=subtract on the decoding load), so the only engine
    work for the MSE stream is the fused Square+accumulate on ACT.
  - u is stored fp16 (verified to move the final loss by <1e-4 rel);
    pass-B element tensors (v, w, z, lv, finv) are fp16, all reductions
    and PSUM accumulations fp32. finv is scaled by 2^17 so ln(v) ~ 0
    and the tiny KL survives the log cancellations.
  - MSE tiles are spread across pass A, the AllReduce window and pass B
    so the DMA stream and the collective latency overlap compute.
"""

import os

import numpy as np

import concourse.bass as bass
import concourse.mybir as mybir
import concourse.tile as tile
from concourse import masks
from concourse.bass_utils import run_bass_kernel_spmd

F32 = mybir.dt.float32
F16 = mybir.dt.float16
AF = mybir.ActivationFunctionType
ALU = mybir.AluOpType
AX = mybir.AxisListType

N, D_IN, D_LAT, K = 200000, 256, 64, 100
ALPHA = 1000.0
NCORES = 8
NROWS = N // NCORES            # 25000 rows per core
G = 4                          # row-groups per supertile
GP = 125                       # rows per group (partition dim)
ST = G * GP                    # 500 rows per supertile
NST = NROWS // ST              # 50 supertiles
GW = D_LAT + 2                 # 66 columns per group: [e (64) | ones | |e|^2]
# MSE stream: per-core flat view [128, 50000] of both X and decoding
MSE_P = 128
MSE_FD = 2000
MSE_T = (NROWS * D_IN) // (MSE_P * MSE_FD)   # 25 tiles
# MSE tile placement: 17 inside pass A, 4 in the AllReduce window, 4 in B
MSE_A = 17
MSE_CC = 4


def _emit_mse_tile(nc, mse_pool, sse_stage, xv, dv, t, use_dma_accum):
    """One [128, MSE_FD] tile of sum((X-dec)^2).

    X loads via HWDGE; decoding loads via SWDGE with accum_op=subtract so
    the tile holds X-dec (or dec-X; the square doesn't care) with zero
    engine work. ACT Square with fused per-partition accumulation."""
    xt = mse_pool.tile([MSE_P, MSE_FD], F32, tag="mse_x")
    nc.sync.dma_start(out=xt[:], in_=xv[:, t * MSE_FD:(t + 1) * MSE_FD])
    if use_dma_accum:
        nc.gpsimd.dma_start(out=xt[:], in_=dv[:, t * MSE_FD:(t + 1) * MSE_FD],
                            accum_op=ALU.subtract)
        diff = xt
    else:
        dt_ = mse_pool.tile([MSE_P, MSE_FD], F32, tag="mse_d")
        nc.sync.dma_start(out=dt_[:], in_=dv[:, t * MSE_FD:(t + 1) * MSE_FD])
        diff = mse_pool.tile([MSE_P, MSE_FD], F32, tag="mse_diff")
        nc.gpsimd.tensor_tensor(diff[:], xt[:], dt_[:], ALU.subtract)
    sq = mse_pool.tile([MSE_P, MSE_FD], F16, tag="mse_sq")
    nc.scalar.activation(sq[:], diff[:], AF.Square,
                         accum_out=sse_stage[:, t:t + 1])


def _body(tc, ctx, x_in, d_in, e_in, c_in, out, parts=("mse", "a", "b")):
    use_dma_accum = "noaccum" not in parts
    nc = tc.nc
    singles = ctx.enter_context(tc.tile_pool(name="singles", bufs=1))
    a_pool = ctx.enter_context(tc.tile_pool(name="passa", bufs=3))
    b_pool = ctx.enter_context(tc.tile_pool(name="passb", bufs=3))
    mse_pool = ctx.enter_context(tc.tile_pool(name="mse", bufs=2))
    ps_pool = ctx.enter_context(tc.tile_pool(name="ps", bufs=2, space="PSUM"))
    ps1_pool = ctx.enter_context(tc.tile_pool(name="ps1", bufs=1, space="PSUM"))
    dram = ctx.enter_context(tc.tile_pool(name="dram", bufs=1, space="DRAM"))

    # ---------------- one-time constants ----------------
    identity16 = singles.tile([GP, GP], F16)
    masks.make_identity(nc, identity16[:])

    c_sb = singles.tile([K, D_LAT], F32)
    nc.sync.dma_start(out=c_sb[:], in_=c_in.ap())
    # c2_j = sum_d centres^2 (per-partition accumulate of Square)
    c_sq = singles.tile([K, D_LAT], F32)
    c2 = singles.tile([K, 1], F32)
    nc.scalar.activation(c_sq[:], c_sb[:], AF.Square, accum_out=c2[:])
    c2p1 = singles.tile([K, 1], F32)
    nc.vector.tensor_scalar_add(c2p1[:], c2[:], 1.0)
    c2p1_16 = singles.tile([K, 1], F16)
    nc.vector.tensor_copy(c2p1_16[:], c2p1[:])
    cm2_16 = singles.tile([K, D_LAT], F16)   # -2 * centres, fp16
    nc.scalar.activation(cm2_16[:], c_sb[:], AF.Copy, scale=-2.0)
    # rhs_aug [66, 100]: rows 0..63 = -2*C^T, row 64 = (c2+1)^T,
    # row 65 = ones. It pairs with per-group lhsT = [e^T; ones; |e|^2^T]
    # so ONE matmul per group yields d2+1 = -2e.c + (c2+1) + |e|^2.
    cmT_ps = ps_pool.tile([D_LAT, GP], F16, tag="tps")
    nc.tensor.transpose(cmT_ps[:, :K], cm2_16[:], identity16[:K, :K])
    c2T_ps = ps_pool.tile([1, K], F16, tag="dot")
    nc.tensor.transpose(c2T_ps[:], c2p1_16[:], identity16[:K, :K])
    c2T_sb = singles.tile([1, K], F16)
    nc.scalar.copy(c2T_sb[:], c2T_ps[:])
    rhs_aug = singles.tile([GW, K], F16)
    nc.scalar.copy(rhs_aug[0:D_LAT, :], cmT_ps[:, :K])
    nc.vector.memset(rhs_aug[D_LAT:D_LAT + 2, :], 1.0)
    # cross-partition move (part 0 -> part 64) via SBUF-to-SBUF DMA
    nc.sync.dma_start(out=rhs_aug[D_LAT:D_LAT + 1, :], in_=c2T_sb[:])

    ones_col = singles.tile([MSE_P, 1], F32)
    nc.vector.memset(ones_col[:], 1.0)

    # persistent stages
    u_store = singles.tile([GP, NST * G * K], F16)        # 40KB/part
    s_stage = singles.tile([GP, NST * G], F32)
    r_stage = singles.tile([GP, NST * G], F32)
    a_stage = singles.tile([GP, NST * G], F32)
    sse_stage = singles.tile([MSE_P, MSE_T], F32)

    f_ps = ps1_pool.tile([1, K], F32, tag="f_ps")

    # flat views for the MSE stream
    xv = x_in.ap().rearrange("a b -> (a b)").rearrange("(p f) -> p f", p=MSE_P)
    dv = d_in.ap().rearrange("a b -> (a b)").rearrange("(p f) -> p f", p=MSE_P)
    # encoding view: supertile st, partition p, free (g, d)
    ev = e_in.ap().rearrange("(s g p) d -> s p g d", g=G, p=GP)

    # ---------------- pass A + interleaved MSE ----------------
    for st in range(NST if "a" in parts else 0):
        # fp32 landing tile; e lands in per-group 66-column blocks so the
        # ones and |e|^2 columns ride along into the transpose.
        e32 = a_pool.tile([GP, G * GW], F32, tag="e32")
        e32_view = bass.AP(
            tensor=e32[:].tensor, offset=e32[:].offset,
            ap=[list(e32[:].ap[0]), [GW, G], [1, D_LAT]])
        nc.sync.dma_start(out=e32_view, in_=ev[st])
        # fp16 working copy (bulk copy incl. the two padding columns;
        # they are overwritten below)
        e_t = a_pool.tile([GP, G * GW], F16, tag="e_t")
        nc.vector.tensor_copy(e_t[:], e32[:])
        e_view = bass.AP(
            tensor=e_t[:].tensor, offset=e_t[:].offset,
            ap=[list(e_t[:].ap[0]), [GW, G], [1, D_LAT]])
        ones_cols = bass.AP(
            tensor=e_t[:].tensor, offset=e_t[:].offset + D_LAT,
            ap=[list(e_t[:].ap[0]), [GW, G], [1, 1]])
        nc.vector.memset(ones_cols, 1.0)
        # |e|^2 per row-group: square (ACT) then blocked reduce (DVE),
        # cast into col 65
        e2 = a_pool.tile([GP, G * D_LAT], F16, tag="e2")
        nc.scalar.activation(
            e2[:].rearrange("p (g d) -> p g d", g=G), e_view, AF.Square)
        e2s = a_pool.tile([GP, G], F32, tag="e2s")
        nc.vector.tensor_reduce(
            e2s[:], e2[:].rearrange("p (g d) -> p g d", g=G),
            axis=AX.X, op=ALU.add)
        se_cols = bass.AP(
            tensor=e_t[:].tensor, offset=e_t[:].offset + D_LAT + 1,
            ap=[list(e_t[:].ap[0]), [GW, G], [1, 1]])
        nc.vector.tensor_copy(se_cols, e2s[:])

        # 4 transposes into one PSUM tile (128-col blocks keep the fp16
        # PSUM writes 4B-aligned), one fp16 copy out
        TB = 128
        tps = ps_pool.tile([GW, G * TB], F16, tag="tps")
        for g in range(G):
            nc.tensor.transpose(
                tps[:, g * TB:g * TB + GP], e_t[:, g * GW:(g + 1) * GW],
                identity16[:GP, :GP])
        eTa = a_pool.tile([GW, G * TB], F16, tag="eTa")
        nc.scalar.copy(eTa[:], tps[:])

        # per group: one fp16 matmul gives d2+1 = -2e.c + (c2+1) + |e|^2
        dot_ps = ps_pool.tile([GP, G * K], F32, tag="dot")
        for g in range(G):
            nc.tensor.matmul(dot_ps[:, g * K:(g + 1) * K],
                             eTa[:, g * TB:g * TB + GP],
                             rhs_aug[:], start=True, stop=True)
        # u = 1/(d2+1): fp32 for the reductions, fp16 for storage
        u32 = a_pool.tile([GP, G * K], F32, tag="u32")
        nc.vector.reciprocal_approx_fast(u32[:], dot_ps[:])
        u16 = u_store[:, st * G * K:(st + 1) * G * K]
        nc.vector.tensor_copy(u16, u32[:])
        # s = sum_j u per group
        nc.vector.tensor_reduce(
            s_stage[:, st * G:(st + 1) * G],
            u32[:].rearrange("p (g k) -> p g k", g=G),
            axis=AX.X, op=ALU.add)
        rs32 = a_pool.tile([GP, G], F32, tag="rs32")
        nc.vector.reciprocal_approx_fast(
            rs32[:], s_stage[:, st * G:(st + 1) * G])
        rs16 = a_pool.tile([GP, G], F16, tag="rs16")
        nc.vector.tensor_copy(rs16[:], rs32[:])
        # f += sum_i u_ij / s_i  (PSUM accumulation across all supertiles)
        for g in range(G):
            nc.tensor.matmul(f_ps[:], rs16[:, g:g + 1],
                             u16[:, g * K:(g + 1) * K],
                             start=(st == 0 and g == 0),
                             stop=(st == NST - 1 and g == G - 1))
        if "mse" in parts and st % 3 == 0 and st // 3 < MSE_A:
            _emit_mse_tile(nc, mse_pool, sse_stage, xv, dv, st // 3,
                           use_dma_accum)

    # ---------------- AllReduce f ----------------
    do_b = "a" in parts and "b" in parts
    do_cc = "a" in parts and "nocc" not in parts
    f_sb = singles.tile([1, K], F32)
    if "a" in parts:
        nc.scalar.copy(f_sb[:], f_ps[:])
    else:
        nc.vector.memset(f_sb[:], 1.0)
    f_part_dram = dram.tile([1, K], F32)
    f_all_dram = dram.tile([1, K], F32)
    nc.sync.dma_start(out=f_part_dram[:], in_=f_sb[:])
    if do_cc:
        nc.gpsimd.collective_compute(
            "AllReduce", ALU.add,
            replica_groups=[list(range(NCORES))],
            ins=[f_part_dram[:].opt()],
            outs=[f_all_dram[:].opt()],
        )
    else:
        nc.sync.dma_start(out=f_all_dram[:], in_=f_part_dram[:])
    # MSE tiles riding the collective's latency window
    if "mse" in parts:
        hi = MSE_A + MSE_CC if "b" in parts else MSE_T
        for t in range(MSE_A if "a" in parts else 0, hi):
            _emit_mse_tile(nc, mse_pool, sse_stage, xv, dv, t, use_dma_accum)

    # broadcast-load f to all partitions AND all 4 group-columns, then
    # finv = 2^17/f. The scale cancels exactly in a/r - ln r, but keeps
    # ln v ~ 0 so the ~2e-6 noise of the fast reciprocal of r is not
    # amplified by |ln v| ~ 12 (which would otherwise bias the tiny KL).
    f_rep = singles.tile([GP, G * K], F32)
    f_ap = f_all_dram[:]
    f_bcast_ap = bass.AP(
        tensor=f_ap.tensor, offset=f_ap.offset,
        ap=[[0, GP], [0, G], [1, K]],
    )
    nc.sync.dma_start(out=f_rep[:], in_=f_bcast_ap)
    finv32 = singles.tile([GP, G * K], F32)
    nc.vector.reciprocal_approx_fast(finv32[:], f_rep[:])
    nc.vector.tensor_scalar_mul(finv32[:], finv32[:], float(2.0 ** 17))
    finv16 = singles.tile([GP, G * K], F16)
    nc.vector.tensor_copy(finv16[:], finv32[:])

    # ---------------- pass B (fp16 elementwise, fp32 reductions) ------
    if not do_b and "a" in parts:
        nc.vector.memset(r_stage[:], 1.0)
        nc.vector.memset(a_stage[:], 1.0)
    for st in range(NST if do_b else 0):
        ub = u_store[:, st * G * K:(st + 1) * G * K]
        v = b_pool.tile([GP, G * K], F16, tag="v")
        nc.gpsimd.tensor_tensor(v[:], ub, finv16[:], ALU.mult)
        lv = b_pool.tile([GP, G * K], F16, tag="lv")
        nc.scalar.activation(lv[:], v[:], AF.Ln)
        w = b_pool.tile([GP, G * K], F16, tag="w")
        nc.gpsimd.tensor_tensor(w[:], v[:], ub, ALU.mult)
        nc.vector.tensor_reduce(
            r_stage[:, st * G:(st + 1) * G],
            w[:].rearrange("p (g k) -> p g k", g=G),
            axis=AX.X, op=ALU.add)
        z = b_pool.tile([GP, G * K], F16, tag="z")
        nc.vector.tensor_tensor(z[:], w[:], lv[:], ALU.mult)
        nc.vector.tensor_reduce(
            a_stage[:, st * G:(st + 1) * G],
            z[:].rearrange("p (g k) -> p g k", g=G),
            axis=AX.X, op=ALU.add)
        if "mse" in parts and st % 10 == 5 and MSE_A + MSE_CC + st // 10 < MSE_T:
            _emit_mse_tile(nc, mse_pool, sse_stage, xv, dv,
                           MSE_A + MSE_CC + st // 10, use_dma_accum)

    if "mse" in parts and not do_b:
        for t in range(MSE_A + MSE_CC if "a" in parts else 0, MSE_T):
            _emit_mse_tile(nc, mse_pool, sse_stage, xv, dv, t, use_dma_accum)
    elif "mse" not in parts:
        nc.vector.memset(sse_stage[:], 0.0)
    if "a" not in parts:
        for t in [s_stage, r_stage, a_stage]:
            nc.vector.memset(t[:], 1.0)

    # ---------------- final combine ----------------
    rr = singles.tile([GP, NST * G], F32)
    nc.vector.reciprocal_approx_fast(rr[:], r_stage[:])
    y = singles.tile([GP, NST * G], F32)
    nc.vector.tensor_tensor(y[:], a_stage[:], rr[:], ALU.mult)
    lnr = singles.tile([GP, NST * G], F32)
    nc.scalar.activation(lnr[:], r_stage[:], AF.Ln)
    lns = singles.tile([GP, NST * G], F32)
    nc.scalar.activation(lns[:], s_stage[:], AF.Ln)
    t1 = singles.tile([GP, NST * G], F32)
    nc.vector.tensor_tensor(t1[:], y[:], lnr[:], ALU.subtract)
    t2 = singles.tile([GP, NST * G], F32)
    nc.vector.tensor_tensor(t2[:], t1[:], lns[:], ALU.add)
    kcol = singles.tile([GP, 1], F32)
    nc.vector.tensor_reduce(kcol[:], t2[:], axis=AX.X, op=ALU.add)
    ssecol = singles.tile([MSE_P, 1], F32)
    nc.vector.tensor_reduce(ssecol[:], sse_stage[:], axis=AX.X, op=ALU.add)
    # partition reduction via matmul with a ones column
    red_ps = ps_pool.tile([1, 2], F32, tag="dot")
    nc.tensor.matmul(red_ps[:, 0:1], ssecol[:], ones_col[:MSE_P, :],
                     start=True, stop=True)
    nc.tensor.matmul(red_ps[:, 1:2], kcol[:], ones_col[:GP, :],
                     start=True, stop=True)
    out_sb = singles.tile([1, 2], F32)
    nc.scalar.copy(out_sb[:], red_ps[:])
    nc.sync.dma_start(out=out.ap(), in_=out_sb[:])


def build_kernel(parts=("mse", "a", "b")):
    from contextlib import ExitStack
    import concourse.bacc as bacc
    nc = bacc.Bacc("TRN2", target_bir_lowering=False, debug=False,
                   num_devices=NCORES)
    x_in = nc.dram_tensor("x_shard", [NROWS, D_IN], F32, kind="ExternalInput")
    d_in = nc.dram_tensor("dec_shard", [NROWS, D_IN], F32,
                          kind="ExternalInput")
    e_in = nc.dram_tensor("enc_shard", [NROWS, D_LAT], F32,
                          kind="ExternalInput")
    c_in = nc.dram_tensor("centres", [K, D_LAT], F32, kind="ExternalInput")
    out = nc.dram_tensor("partials", [1, 2], F32, kind="ExternalOutput")
    with tile.TileContext(nc) as tc:
        with ExitStack() as ctx:
            _body(tc, ctx, x_in, d_in, e_in, c_in, out, parts=parts)
    nc.compile()
    return nc


_built = None
LAST_RESULTS = None


def _get_built():
    global _built
    if _built is None:
        parts = tuple(os.environ.get("KERNEL_PARTS", "mse,a,b,noaccum").split(","))
        _built = build_kernel(parts)
    return _built


def kernel(X, encoding, decoding, centres):
    X = np.ascontiguousarray(np.asarray(X, dtype=np.float32))
    encoding = np.ascontiguousarray(np.asarray(encoding, dtype=np.float32))
    decoding = np.ascontiguousarray(np.asarray(decoding, dtype=np.float32))
    centres = np.ascontiguousarray(np.asarray(centres, dtype=np.float32))

    nc = _get_built()
    in_maps = []
    for c in range(NCORES):
        sl = slice(c * NROWS, (c + 1) * NROWS)
        in_maps.append({
            "x_shard": X[sl],
            "dec_shard": decoding[sl],
            "enc_shard": encoding[sl],
            "centres": centres,
        })
    res = run_bass_kernel_spmd(nc, in_maps, core_ids=list(range(NCORES)))
    global LAST_RESULTS
    LAST_RESULTS = res
    sse = 0.0
    kl_sum = 0.0
    for r in res.results:
        sse += float(r["partials"][0, 0])
        kl_sum += float(r["partials"][0, 1])
    loss = sse / (N * D_IN) + ALPHA * (kl_sum / N)
    return np.float32(loss)
